# revision 49
# baseline (speedup 1.0000x reference)
"""Trainium2 Bass kernel for nn_MultiHeadModulator (8-core SPMD).

Math reformulation (exact): with a single query q = Wq@z_curr+bq,
  - dot scores:  score[l,h] = z[l]·A[:,h] + c[h],   A[:,h] = Wk[hb,:]^T @ q[hb]
  - rel scores fold into a per-(l,h) additive bias known on the host
  - value sum:   sum_l e[l,h]*v[l] = Wv @ (sum_l e[l,h]*z[l]) + (sum_l e[l,h])*bv
so the device only computes, per L-shard:
  score^T = A^T z^T   (PE, fp8 DoubleRow),  e^T = exp(scale*score + c_h) * fac
  U[h,:] += e^T z     (PE, fp8 DoubleRow),  S[h] from exp's accum_out
and the host applies Wv/Wo and the softmax normalization to the tiny [8,512]
all-core sums.  Softmax runs without max-subtraction: scores are O(1) by
construction (validated |score| < 3).

Sharding: z_past split into 8 contiguous shards of 8192 rows, one per core.
The host ships each shard twice (feature-major for scores, row-major for U)
in fp8, pre-packed for DoubleRow access patterns (the dual layout costs 2x
HBM but avoids any on-chip transpose of z; only the tiny e^T [8,512] tiles
get PE-transposed per block).

Measured: 39.83/41.27 us HW exec (8 cores), rel err 5.8e-3 vs the f32
reference (the original unpipelined schedule measured 41.6-45.0).

Schedule (perfetto-derived):
  - ALL bulk loads ride the sync HWDGE ring as 13 UNIFORM ~0.75MB D2D
    triggers: a front chunk carrying zt block 0 + a_dr + zt block 1,
    cst, then 3-block zt/zn chunks interleaved in consumption order
    with zn lagging zt by one chunk.  The sync DGE round-robins among
    outstanding D2D requests, so with a few LARGE triggers every
    completion semaphore fires late (block-4 scores once ran 6us after
    its bytes had landed); uniform modest chunks make completion order
    track issue order.  Each chunk completion carries a fixed ~0.4us
    semaphore overhead, so 13 chunks beat 18 (2-block chunks measured
    42.2-42.5) and 35 (dispatch-paced) - the stream runs at the ring's
    ~420GB/s peak, all 8.4MB landed by ~27.5us, PE gap total 3.1us.
    Fanning bulk to the scalar HWDGE ring (~35-55GB/s) or gpsimd SWDGE
    (~115GB/s, ~11us boot) actively STEALS sync throughput (sync dropped
    8x while SWDGE streamed); every multi-ring variant measured 5-8us
    slower (44.7-51).
  - the compute loop is software-pipelined two deep; each PE iteration is
    [scores(b) | transposes(b-1) | U-matmuls(b-2)] so the ACT exp and the
    DVE e8-cast of a block never block the PE stream.  A dense PE stream
    also keeps the tensor engine's DVFS p-state ramped (0.65 -> 1.2 ->
    2.4GHz after 3us continuous; matmuls drop 379ns -> 216ns per 512-col
    fp8-DR stream).  Coarser 2-block stages and finer front D2Ds both
    measured ~1.5-3us worse.
  - the ACT chain stays pure exp: S partials come from per-block DVE
    tensor_reduce (the exp accum_out's 185ns ACTIVATION_READ_ACCUMULATOR
    follower serialized the ACT chain; plain tensor_reduce is HW-safe,
    tensor_tensor_reduce is NOT - it crashes on HW, fine in CoreSim).
  - weight-side DoubleRow LDWEIGHTS requires the pair-dim step to be a
    multiple of 16 elements (a_dr lives in zt0 cols 512:528; 528=33*16).
  - cb ships as bf16 inside the const tile (a per-head-constant bias
    error cancels in the U/S softmax ratio).
  - PSUM budget (8 banks): 5x score + 2x e-transpose + 1x U accumulator.
  - remaining fixed costs: ~8.7us until first DMA byte (engine boot +
    NEFF preamble), ~5us tail after the last U matmul (PSUM copy + out
    DMA + end-of-kernel multi-engine fence).
"""

import numpy as np
import ml_dtypes

import concourse.bass as bass  # noqa: F401  (engine namespaces live on the nc)
import concourse.mybir as mybir
import concourse.tile as tile
from concourse import bacc
from concourse.bass_utils import run_bass_kernel_spmd

HEADS = 8
REL_MAX = 64
DIM = 256
D2 = 512                      # flattened real feature dim
HD = DIM // HEADS             # 32 complex => 64 reals per head block
L_TOTAL = 65536
N_CORES = 8
L_SHARD = L_TOTAL // N_CORES  # 8192
N_BLOCKS = L_SHARD // 512     # 16 blocks of 512 rows
BLK_PER_SUPER = 4             # blocks per DMA (1 MB chunks)
N_SUPER = N_BLOCKS // BLK_PER_SUPER
SCALE = 1.0 / np.sqrt(HD)

FP8 = ml_dtypes.float8_e4m3   # == mybir.dt.float8e4 (trainium E4M3, max 240)
BF16 = ml_dtypes.bfloat16

TRACE = False                 # test.py can flip this for profiling runs
TRACE_KW = {}

_cached = {}


def _build_program(full_fac: bool):
    nc = bacc.Bacc(
        "TRN2", target_bir_lowering=False, debug=False, num_devices=N_CORES
    )
    DR = mybir.MatmulPerfMode.DoubleRow
    f8 = mybir.dt.float8e4

    facw = L_SHARD if full_fac else 512
    # front chunk: zt block 0, a_dr at cols 512:528 (pair-dim step stays a
    # multiple of 16: 1040 = 65*16), zt block 1 at cols 528:1040
    ZT0 = nc.dram_tensor("zt0", [128, 2, 2, 1040], f8, kind="ExternalInput")
    # ~0.75MB chunks (13 triggers total): each per-chunk completion carries
    # a fixed ~0.37us semaphore/round-robin overhead, so fewer uniform
    # chunks raise effective stream rate; zt/zn interleaved in consumption
    # order with zn lagging one chunk
    ZT3 = nc.dram_tensor(
        "zt3", [4, 128, 3, 2, 2, 512], f8, kind="ExternalInput"
    )
    ZT1415 = nc.dram_tensor(
        "zt1415", [128, 2, 2, 2, 512], f8, kind="ExternalInput"
    )
    ZN3 = nc.dram_tensor(
        "zn3", [4, 128, 3, 2, 2, 512], f8, kind="ExternalInput"
    )
    ZN1213 = nc.dram_tensor(
        "zn1213", [128, 2, 2, 2, 512], f8, kind="ExternalInput"
    )
    ZN1415 = nc.dram_tensor(
        "zn1415", [128, 2, 2, 2, 512], f8, kind="ExternalInput"
    )
    # col 0: cb (bf16; per-head-constant bias error cancels in U/S),
    # cols 1:9 identity, cols 9:9+facw rel-bias correction factors
    CST = nc.dram_tensor("cst", [8, 9 + facw], mybir.dt.bfloat16,
                         kind="ExternalInput")
    OUT_U = nc.dram_tensor("out_u", [8, 512], mybir.dt.float32,
                           kind="ExternalOutput")
    OUT_S = nc.dram_tensor("out_s", [8, N_BLOCKS], mybir.dt.float32,
                           kind="ExternalOutput")

    with tile.TileContext(nc) as tc:
        with (
            tc.tile_pool(name="zt", bufs=1) as zt_pool,
            tc.tile_pool(name="zn", bufs=1) as zn_pool,
            tc.tile_pool(name="consts", bufs=1) as const_pool,
            tc.tile_pool(name="et", bufs=8) as et_pool,
            tc.tile_pool(name="e8", bufs=8) as e8_pool,
            tc.tile_pool(name="outs", bufs=1) as out_pool,
            tc.tile_pool(name="ps_sc", bufs=5, space="PSUM") as sc_pool,
            tc.tile_pool(name="ps_etp", bufs=2, space="PSUM") as etp_pool,
            tc.tile_pool(name="ps_acc", bufs=1, space="PSUM") as acc_pool,
        ):
            # ~10 big D2D triggers, all on the sync HWDGE ring (sustains
            # ~420GB/s when fed large requests; all 8.4MB lands by ~25-30us)
            zt0_sb = const_pool.tile([128, 2, 2, 1040], f8)
            nc.sync.dma_start(zt0_sb[:], ZT0[:])
            cst_sb = const_pool.tile([8, 9 + facw], mybir.dt.bfloat16)
            nc.sync.dma_start(cst_sb[:], CST[:])
            zt3_tiles = [
                zt_pool.tile([128, 3, 2, 2, 512], f8, name=f"zt3_{s}")
                for s in range(4)
            ]
            zt1415_sb = zt_pool.tile([128, 2, 2, 2, 512], f8)
            zn3_tiles = [
                zn_pool.tile([128, 3, 2, 2, 512], f8, name=f"zn3_{s}")
                for s in range(4)
            ]
            zn1213_sb = zn_pool.tile([128, 2, 2, 2, 512], f8)
            zn1415_sb = zn_pool.tile([128, 2, 2, 2, 512], f8)
            # zn lags zt by one chunk, matching the 2-deep pipeline's
            # consumption order (sc_b needs zt_b; U_{b-2} needs zn_{b-2})
            for s in range(4):
                nc.sync.dma_start(zt3_tiles[s][:], ZT3[s])
                nc.sync.dma_start(zn3_tiles[s][:], ZN3[s])
            nc.sync.dma_start(zt1415_sb[:], ZT1415[:])
            nc.sync.dma_start(zn1213_sb[:], ZN1213[:])
            nc.sync.dma_start(zn1415_sb[:], ZN1415[:])

            u_ps = acc_pool.tile([8, 512], mybir.dt.float32)
            outs_sb = out_pool.tile([8, N_BLOCKS], mybir.dt.float32)
            u_sb = out_pool.tile([8, 512], mybir.dt.float32)

            def zt_view(b):
                if b == 0:
                    return zt0_sb[:, :, :, 0:512]
                if b == 1:
                    return zt0_sb[:, :, :, 528:1040]
                if b >= 14:
                    return zt1415_sb[:, b - 14]
                return zt3_tiles[(b - 2) // 3][:, (b - 2) % 3]

            def zn_view(b):
                if b >= 14:
                    return zn1415_sb[:, b - 14]
                if b >= 12:
                    return zn1213_sb[:, b - 12]
                return zn3_tiles[b // 3][:, b % 3]

            def scores(b):
                # score^T[h, l] for the block's 512 rows, K=512 via 2x DoubleRow
                zt_t = zt_view(b)
                sc = sc_pool.tile(
                    [8, 512], mybir.dt.float32, tag="sc", name=f"sc_{b}"
                )
                for cpair in range(2):
                    nc.tensor.matmul(
                        sc[:],
                        zt0_sb[:, cpair, :, 512:520],
                        zt_t[:, cpair] if b else zt0_sb[:, cpair, :, 0:512],
                        start=(cpair == 0),
                        stop=(cpair == 1),
                        perf_mode=DR,
                    )
                et = et_pool.tile(
                    [8, 512], mybir.dt.bfloat16, tag="et", name=f"et_{b}"
                )
                # S via the exp's accumulator for uncorrected blocks: the
                # ACT chain has ~0.7us/block slack now that the PE paces,
                # and this keeps the DVE to casts only (bunched DVE
                # reduces at chunk boundaries can delay the e8 casts)
                accum = (
                    {}
                    if (full_fac or b == 0)
                    else {"accum_out": outs_sb[:, b : b + 1]}
                )
                nc.scalar.activation(
                    et[:],
                    sc[:],
                    mybir.ActivationFunctionType.Exp,
                    bias=cst_sb[:, 0:1],
                    scale=float(SCALE),
                    **accum,
                )
                # rel-bias correction factors: only block 0 deviates from 1
                # in the common curr_pos regime (full_fac covers the rest)
                if full_fac or b == 0:
                    etc = et_pool.tile(
                        [8, 512], mybir.dt.bfloat16, tag="etc", name=f"etc_{b}"
                    )
                    nc.vector.tensor_mul(
                        etc[:], et[:], cst_sb[:, 9 + 512 * b : 9 + 512 * (b + 1)]
                    )
                    nc.vector.tensor_reduce(
                        outs_sb[:, b : b + 1],
                        etc[:],
                        axis=mybir.AxisListType.X,
                        op=mybir.AluOpType.add,
                    )
                else:
                    etc = et
                return etc

            def transposes(b, etc):
                # transpose e^T -> e[l,h] in 4x [8,128] chunks (PE+identity)
                etp = etp_pool.tile(
                    [128, 4, 8], mybir.dt.bfloat16, tag="etp", name=f"etp_{b}"
                )
                for quad in range(4):
                    nc.tensor.transpose(
                        etp[:, quad],
                        etc[:, 128 * quad : 128 * (quad + 1)],
                        cst_sb[:, 1:9],
                    )
                e8 = e8_pool.tile([128, 4, 16], f8, tag="e8", name=f"e8_{b}")
                nc.vector.tensor_copy(e8[:, :, 0:8], etp[:])
                return e8

            def weighted_sum(b, e8, first, last):
                zn_t = zn_view(b)
                for s in range(2):
                    nc.tensor.matmul(
                        u_ps[:],
                        e8[:, 2 * s : 2 * s + 2, 0:8],
                        zn_t[:, s],
                        start=(first and s == 0),
                        stop=(last and s == 1),
                        perf_mode=DR,
                    )

            # two-deep software pipeline; each PE iteration runs
            #   [scores(b) | transposes(b-1) | U-matmul(b-2)]
            # so exp(b-1) hides under scores(b) and the DVE e8-cast of a
            # block never blocks the PE stream (keeps the DVFS p-state up)
            e8s = {}
            etcs = {}
            for b in range(N_BLOCKS):
                etcs[b] = scores(b)
                if b >= 1:
                    e8s[b - 1] = transposes(b - 1, etcs.pop(b - 1))
                if b >= 2:
                    weighted_sum(b - 2, e8s.pop(b - 2), b == 2, False)
            e8s[N_BLOCKS - 1] = transposes(
                N_BLOCKS - 1, etcs.pop(N_BLOCKS - 1)
            )
            weighted_sum(N_BLOCKS - 2, e8s.pop(N_BLOCKS - 2), False, False)
            weighted_sum(N_BLOCKS - 1, e8s.pop(N_BLOCKS - 1), False, True)

            # S partials ride the idle sync ring; ACT (closest to PSUM,
            # free after the last exp) copies U, sync sends it
            nc.sync.dma_start(OUT_S[:], outs_sb[:])
            nc.scalar.copy(u_sb[:], u_ps[:])
            nc.sync.dma_start(OUT_U[:], u_sb[:])

    nc.compile()
    return nc


def _get_program(full_fac: bool):
    if full_fac not in _cached:
        _cached[full_fac] = _build_program(full_fac)
    return _cached[full_fac]


def kernel(curr_pos, z_curr, z_past, Wq, bq, Wk, bk, Wv, bv, Wo, bo, rel_bias):
    curr_pos = int(np.asarray(curr_pos))
    z_curr = np.asarray(z_curr, dtype=np.float32)
    z_past = np.asarray(z_past, dtype=np.float32)
    Wq = np.asarray(Wq, dtype=np.float32)
    bq = np.asarray(bq, dtype=np.float32)
    Wk = np.asarray(Wk, dtype=np.float32)
    bk = np.asarray(bk, dtype=np.float32)
    Wv = np.asarray(Wv, dtype=np.float32)
    bv = np.asarray(bv, dtype=np.float32)
    Wo = np.asarray(Wo, dtype=np.float32)
    bo = np.asarray(bo, dtype=np.float32)
    rel_bias = np.asarray(rel_bias, dtype=np.float32)

    # ---- host-side O(D^2) prep (f64) ----
    q = z_curr.reshape(-1).astype(np.float64) @ Wq.T.astype(np.float64) + bq
    A = np.zeros((D2, HEADS), np.float64)
    c = np.zeros(HEADS, np.float64)
    for h in range(HEADS):
        sl = slice(h * 2 * HD, (h + 1) * 2 * HD)
        A[:, h] = Wk[sl, :].T.astype(np.float64) @ q[sl]
        c[h] = bk[sl].astype(np.float64) @ q[sl]
    relflat = rel_bias.reshape(2 * REL_MAX + 1, D2).astype(np.float64)
    rb = np.stack(
        [
            relflat[:, h * 2 * HD : (h + 1) * 2 * HD] @ q[h * 2 * HD : (h + 1) * 2 * HD]
            for h in range(HEADS)
        ],
        axis=1,
    )  # [129, 8]
    idx = np.clip(
        curr_pos - L_TOTAL + np.arange(L_TOTAL) + REL_MAX, 0, 2 * REL_MAX
    ).astype(np.int64)

    z8 = np.clip(z_past.reshape(L_TOTAL, D2), -240.0, 240.0).astype(FP8)
    A8 = np.clip(A, -240.0, 240.0).astype(np.float32).astype(FP8)
    a_dr = np.zeros((128, 2, 2, 16), FP8)
    a_dr[:, :, :, 0:8] = A8.reshape(2, 2, 128, HEADS).transpose(2, 0, 1, 3)

    in_maps = []
    facs = []
    for core in range(N_CORES):
        zc = z8[core * L_SHARD : (core + 1) * L_SHARD]
        # zt_b[p, cpair, d, l] = zc[512*b + l, 256*cpair + 128*d + p]
        zt_all = np.ascontiguousarray(
            zc.reshape(N_BLOCKS, 512, 2, 2, 128).transpose(0, 4, 2, 3, 1)
        )
        # zn_b[p, s, d, f] = zc[512*b + 256*s + 128*d + p, f]
        zn_all = np.ascontiguousarray(
            zc.reshape(N_BLOCKS, 2, 2, 128, 512).transpose(0, 3, 1, 2, 4)
        )

        def pk(blob, lo, hi):  # [nblk,128,2,2,512] -> [128,nblk,2,2,512]
            return np.ascontiguousarray(blob[lo:hi].transpose(1, 0, 2, 3, 4))

        def pks(blob, lo, hi):  # -> [nsup,128,4,2,2,512]
            n = (hi - lo) // 4
            return np.ascontiguousarray(
                blob[lo:hi].reshape(n, 4, 128, 2, 2, 512).transpose(
                    0, 2, 1, 3, 4, 5
                )
            )

        idx_c = idx[core * L_SHARD : (core + 1) * L_SHARD]
        base = int(np.bincount(idx_c, minlength=2 * REL_MAX + 1).argmax())
        cb = ((c + rb[base]) * SCALE).astype(np.float32).reshape(HEADS, 1)
        fac = np.ascontiguousarray(
            np.exp((rb[idx_c] - rb[base]) * SCALE).T.astype(BF16)
        )
        facs.append(fac)
        in_maps.append(
            {
                "zt0": np.concatenate(
                    [zt_all[0], a_dr, zt_all[1]], axis=3
                ),
                "zt3": np.ascontiguousarray(
                    zt_all[2:14].reshape(4, 3, 128, 2, 2, 512).transpose(
                        0, 2, 1, 3, 4, 5
                    )
                ),
                "zt1415": np.ascontiguousarray(
                    zt_all[14:16].transpose(1, 0, 2, 3, 4)
                ),
                "zn3": np.ascontiguousarray(
                    zn_all[0:12].reshape(4, 3, 128, 2, 2, 512).transpose(
                        0, 2, 1, 3, 4, 5
                    )
                ),
                "zn1213": np.ascontiguousarray(
                    zn_all[12:14].transpose(1, 0, 2, 3, 4)
                ),
                "zn1415": np.ascontiguousarray(
                    zn_all[14:16].transpose(1, 0, 2, 3, 4)
                ),
                "cb": cb,
            }
        )

    # fast path: correction factors are 1.0 outside block 0 on every core
    full_fac = any(
        not np.all(f[:, 512:] == np.asarray(1.0, BF16)) for f in facs
    )
    facw = L_SHARD if full_fac else 512
    for core, m in enumerate(in_maps):
        cst = np.zeros((8, 9 + facw), BF16)
        cst[:, 0:1] = m.pop("cb").astype(BF16)
        cst[:, 1:9] = np.eye(8, dtype=BF16)
        cst[:, 9:] = facs[core][:, 0:facw]
        m["cst"] = cst
    nc = _get_program(full_fac)
    res = run_bass_kernel_spmd(
        nc, in_maps, list(range(N_CORES)), trace=TRACE, **TRACE_KW
    )
    if TRACE:
        kernel.last_result = res

    U = np.zeros((HEADS, D2), np.float64)
    S = np.zeros(HEADS, np.float64)
    for r in res.results:
        U += np.asarray(r["out_u"], dtype=np.float64)
        S += np.asarray(r["out_s"], dtype=np.float64).sum(axis=1)

    hvec = np.zeros(D2, np.float64)
    for h in range(HEADS):
        sl = slice(h * 2 * HD, (h + 1) * 2 * HD)
        hvec[sl] = Wv[sl, :].astype(np.float64) @ (U[h] / S[h]) + bv[sl]
    out = hvec @ Wo.T.astype(np.float64) + bo
    return out.reshape(DIM, 2).astype(np.float32)


# revision 50
# speedup vs baseline: 1.0665x; 1.0665x over previous
"""Trainium2 Bass kernel for nn_MultiHeadModulator (8-core SPMD).

Math reformulation (exact): with a single query q = Wq@z_curr+bq,
  - dot scores:  score[l,h] = z[l]·A[:,h] + c[h],   A[:,h] = Wk[hb,:]^T @ q[hb]
  - rel scores fold into a per-(l,h) additive bias known on the host
  - value sum:   sum_l e[l,h]*v[l] = Wv @ (sum_l e[l,h]*z[l]) + (sum_l e[l,h])*bv
so the device only computes, per L-shard:
  score^T = A^T z^T   (PE, fp8 DoubleRow),  e^T = exp(scale*score + c_h) * fac
  U[h,:] += e^T z     (PE, fp8 DoubleRow),  S[h] from exp's accum_out
and the host applies Wv/Wo and the softmax normalization to the tiny [8,512]
all-core sums.  Softmax runs without max-subtraction: scores are O(1) by
construction (validated |score| < 3).

Sharding: z_past split into 8 contiguous shards of 8192 rows, one per core.
The host ships each shard twice (feature-major for scores, row-major for U)
in fp8, pre-packed for DoubleRow access patterns (the dual layout costs 2x
HBM but avoids any on-chip transpose of z; only the tiny e^T [8,512] tiles
get PE-transposed per block).

Measured: 39.83/41.27 us HW exec (8 cores), rel err 5.8e-3 vs the f32
reference (the original unpipelined schedule measured 41.6-45.0).

Schedule (perfetto-derived):
  - ALL bulk loads ride the sync HWDGE ring as 13 UNIFORM ~0.75MB D2D
    triggers: a front chunk carrying zt block 0 + a_dr + zt block 1,
    cst, then 3-block zt/zn chunks interleaved in consumption order
    with zn lagging zt by one chunk.  The sync DGE round-robins among
    outstanding D2D requests, so with a few LARGE triggers every
    completion semaphore fires late (block-4 scores once ran 6us after
    its bytes had landed); uniform modest chunks make completion order
    track issue order.  Each chunk completion carries a fixed ~0.4us
    semaphore overhead, so 13 chunks beat 18 (2-block chunks measured
    42.2-42.5) and 35 (dispatch-paced) - the stream runs at the ring's
    ~420GB/s peak, all 8.4MB landed by ~27.5us, PE gap total 3.1us.
    Fanning bulk to the scalar HWDGE ring (~35-55GB/s) or gpsimd SWDGE
    (~115GB/s, ~11us boot) actively STEALS sync throughput (sync dropped
    8x while SWDGE streamed); every multi-ring variant measured 5-8us
    slower (44.7-51).
  - the compute loop is software-pipelined two deep; each PE iteration is
    [scores(b) | transposes(b-1) | U-matmuls(b-2)] so the ACT exp and the
    DVE e8-cast of a block never block the PE stream.  A dense PE stream
    also keeps the tensor engine's DVFS p-state ramped (0.65 -> 1.2 ->
    2.4GHz after 3us continuous; matmuls drop 379ns -> 216ns per 512-col
    fp8-DR stream).  Coarser 2-block stages and finer front D2Ds both
    measured ~1.5-3us worse.
  - the ACT chain stays pure exp: S partials come from per-block DVE
    tensor_reduce (the exp accum_out's 185ns ACTIVATION_READ_ACCUMULATOR
    follower serialized the ACT chain; plain tensor_reduce is HW-safe,
    tensor_tensor_reduce is NOT - it crashes on HW, fine in CoreSim).
  - weight-side DoubleRow LDWEIGHTS requires the pair-dim step to be a
    multiple of 16 elements (a_dr lives in zt0 cols 512:528; 528=33*16).
  - cb ships as bf16 inside the const tile (a per-head-constant bias
    error cancels in the U/S softmax ratio).
  - PSUM budget (8 banks): 5x score + 2x e-transpose + 1x U accumulator.
  - remaining fixed costs: ~8.7us until first DMA byte (engine boot +
    NEFF preamble), ~5us tail after the last U matmul (PSUM copy + out
    DMA + end-of-kernel multi-engine fence).
"""

import numpy as np
import ml_dtypes

import concourse.bass as bass  # noqa: F401  (engine namespaces live on the nc)
import concourse.mybir as mybir
import concourse.tile as tile
from concourse import bacc
from concourse.bass_utils import run_bass_kernel_spmd

HEADS = 8
REL_MAX = 64
DIM = 256
D2 = 512                      # flattened real feature dim
HD = DIM // HEADS             # 32 complex => 64 reals per head block
L_TOTAL = 65536
N_CORES = 8
L_SHARD = L_TOTAL // N_CORES  # 8192
N_BLOCKS = L_SHARD // 512     # 16 blocks of 512 rows
BLK_PER_SUPER = 4             # blocks per DMA (1 MB chunks)
N_SUPER = N_BLOCKS // BLK_PER_SUPER
SCALE = 1.0 / np.sqrt(HD)

FP8 = ml_dtypes.float8_e4m3   # == mybir.dt.float8e4 (trainium E4M3, max 240)
BF16 = ml_dtypes.bfloat16

TRACE = False                 # test.py can flip this for profiling runs
TRACE_KW = {}

_cached = {}


def _build_program(full_fac: bool):
    nc = bacc.Bacc(
        "TRN2", target_bir_lowering=False, debug=False, num_devices=N_CORES
    )
    DR = mybir.MatmulPerfMode.DoubleRow
    f8 = mybir.dt.float8e4

    facw = L_SHARD if full_fac else 512
    # front chunk: zt block 0, a_dr at cols 512:528 (pair-dim step stays a
    # multiple of 16: 1040 = 65*16), zt block 1 at cols 528:1040
    ZT0 = nc.dram_tensor("zt0", [128, 2, 2, 1040], f8, kind="ExternalInput")
    # ~0.75MB chunks (13 triggers total): each per-chunk completion carries
    # a fixed ~0.37us semaphore/round-robin overhead, so fewer uniform
    # chunks raise effective stream rate; zt/zn interleaved in consumption
    # order with zn lagging one chunk
    ZT3 = nc.dram_tensor(
        "zt3", [4, 128, 3, 2, 2, 512], f8, kind="ExternalInput"
    )
    ZT1415 = nc.dram_tensor(
        "zt1415", [128, 2, 2, 2, 512], f8, kind="ExternalInput"
    )
    ZN3 = nc.dram_tensor(
        "zn3", [4, 128, 3, 2, 2, 512], f8, kind="ExternalInput"
    )
    ZN1213 = nc.dram_tensor(
        "zn1213", [128, 2, 2, 2, 512], f8, kind="ExternalInput"
    )
    ZN1415 = nc.dram_tensor(
        "zn1415", [128, 2, 2, 2, 512], f8, kind="ExternalInput"
    )
    # col 0: cb (bf16; per-head-constant bias error cancels in U/S),
    # cols 1:9 identity, cols 9:9+facw rel-bias correction factors
    CST = nc.dram_tensor("cst", [8, 9 + facw], mybir.dt.bfloat16,
                         kind="ExternalInput")
    OUT_U = nc.dram_tensor("out_u", [8, 512], mybir.dt.float32,
                           kind="ExternalOutput")
    OUT_S = nc.dram_tensor("out_s", [8, N_BLOCKS], mybir.dt.float32,
                           kind="ExternalOutput")

    with tile.TileContext(nc) as tc:
        with (
            tc.tile_pool(name="zt", bufs=1) as zt_pool,
            tc.tile_pool(name="zn", bufs=1) as zn_pool,
            tc.tile_pool(name="consts", bufs=1) as const_pool,
            tc.tile_pool(name="et", bufs=8) as et_pool,
            tc.tile_pool(name="e8", bufs=8) as e8_pool,
            tc.tile_pool(name="outs", bufs=1) as out_pool,
            tc.tile_pool(name="ps_sc", bufs=5, space="PSUM") as sc_pool,
            tc.tile_pool(name="ps_etp", bufs=2, space="PSUM") as etp_pool,
            tc.tile_pool(name="ps_acc", bufs=1, space="PSUM") as acc_pool,
        ):
            # ~10 big D2D triggers, all on the sync HWDGE ring (sustains
            # ~420GB/s when fed large requests; all 8.4MB lands by ~25-30us)
            zt0_sb = const_pool.tile([128, 2, 2, 1040], f8)
            nc.sync.dma_start(zt0_sb[:], ZT0[:])
            cst_sb = const_pool.tile([8, 9 + facw], mybir.dt.bfloat16)
            nc.sync.dma_start(cst_sb[:], CST[:])
            zt3_tiles = [
                zt_pool.tile([128, 3, 2, 2, 512], f8, name=f"zt3_{s}")
                for s in range(4)
            ]
            zt1415_sb = zt_pool.tile([128, 2, 2, 2, 512], f8)
            zn3_tiles = [
                zn_pool.tile([128, 3, 2, 2, 512], f8, name=f"zn3_{s}")
                for s in range(4)
            ]
            zn1213_sb = zn_pool.tile([128, 2, 2, 2, 512], f8)
            zn1415_sb = zn_pool.tile([128, 2, 2, 2, 512], f8)
            # zn lags zt by one chunk, matching the 2-deep pipeline's
            # consumption order (sc_b needs zt_b; U_{b-2} needs zn_{b-2})
            for s in range(4):
                nc.sync.dma_start(zt3_tiles[s][:], ZT3[s])
                nc.sync.dma_start(zn3_tiles[s][:], ZN3[s])
            nc.sync.dma_start(zt1415_sb[:], ZT1415[:])
            nc.sync.dma_start(zn1213_sb[:], ZN1213[:])
            nc.sync.dma_start(zn1415_sb[:], ZN1415[:])

            u_ps = acc_pool.tile([8, 512], mybir.dt.float32)
            outs_sb = out_pool.tile([8, N_BLOCKS], mybir.dt.float32)
            u_sb = out_pool.tile([8, 512], mybir.dt.float32)

            def zt_view(b):
                if b == 0:
                    return zt0_sb[:, :, :, 0:512]
                if b == 1:
                    return zt0_sb[:, :, :, 528:1040]
                if b >= 14:
                    return zt1415_sb[:, b - 14]
                return zt3_tiles[(b - 2) // 3][:, (b - 2) % 3]

            def zn_view(b):
                if b >= 14:
                    return zn1415_sb[:, b - 14]
                if b >= 12:
                    return zn1213_sb[:, b - 12]
                return zn3_tiles[b // 3][:, b % 3]

            def scores(b):
                # score^T[h, l] for the block's 512 rows, K=512 via 2x DoubleRow
                zt_t = zt_view(b)
                sc = sc_pool.tile(
                    [8, 512], mybir.dt.float32, tag="sc", name=f"sc_{b}"
                )
                for cpair in range(2):
                    nc.tensor.matmul(
                        sc[:],
                        zt0_sb[:, cpair, :, 512:520],
                        zt_t[:, cpair] if b else zt0_sb[:, cpair, :, 0:512],
                        start=(cpair == 0),
                        stop=(cpair == 1),
                        perf_mode=DR,
                    )
                et = et_pool.tile(
                    [8, 512], mybir.dt.bfloat16, tag="et", name=f"et_{b}"
                )
                nc.scalar.activation(
                    et[:],
                    sc[:],
                    mybir.ActivationFunctionType.Exp,
                    bias=cst_sb[:, 0:1],
                    scale=float(SCALE),
                )
                # rel-bias correction factors: only block 0 deviates from 1
                # in the common curr_pos regime (full_fac covers the rest)
                if full_fac or b == 0:
                    etc = et_pool.tile(
                        [8, 512], mybir.dt.bfloat16, tag="etc", name=f"etc_{b}"
                    )
                    nc.vector.tensor_mul(
                        etc[:], et[:], cst_sb[:, 9 + 512 * b : 9 + 512 * (b + 1)]
                    )
                else:
                    etc = et
                # S on the idle DVE so the ACT chain stays pure exp (no
                # 185ns ACTIVATION_READ_ACCUMULATOR serializing it)
                nc.vector.tensor_reduce(
                    outs_sb[:, b : b + 1],
                    etc[:],
                    axis=mybir.AxisListType.X,
                    op=mybir.AluOpType.add,
                )
                return etc

            def transposes(b, etc):
                # transpose e^T -> e[l,h] in 4x [8,128] chunks (PE+identity)
                etp = etp_pool.tile(
                    [128, 4, 8], mybir.dt.bfloat16, tag="etp", name=f"etp_{b}"
                )
                for quad in range(4):
                    nc.tensor.transpose(
                        etp[:, quad],
                        etc[:, 128 * quad : 128 * (quad + 1)],
                        cst_sb[:, 1:9],
                    )
                e8 = e8_pool.tile([128, 4, 16], f8, tag="e8", name=f"e8_{b}")
                nc.vector.tensor_copy(e8[:, :, 0:8], etp[:])
                return e8

            def weighted_sum(b, e8, first, last):
                zn_t = zn_view(b)
                for s in range(2):
                    nc.tensor.matmul(
                        u_ps[:],
                        e8[:, 2 * s : 2 * s + 2, 0:8],
                        zn_t[:, s],
                        start=(first and s == 0),
                        stop=(last and s == 1),
                        perf_mode=DR,
                    )

            # two-deep software pipeline; each PE iteration runs
            #   [scores(b) | transposes(b-1) | U-matmul(b-2)]
            # so exp(b-1) hides under scores(b) and the DVE e8-cast of a
            # block never blocks the PE stream (keeps the DVFS p-state up)
            e8s = {}
            etcs = {}
            for b in range(N_BLOCKS):
                etcs[b] = scores(b)
                if b >= 1:
                    e8s[b - 1] = transposes(b - 1, etcs.pop(b - 1))
                if b >= 2:
                    weighted_sum(b - 2, e8s.pop(b - 2), b == 2, False)
            e8s[N_BLOCKS - 1] = transposes(
                N_BLOCKS - 1, etcs.pop(N_BLOCKS - 1)
            )
            weighted_sum(N_BLOCKS - 2, e8s.pop(N_BLOCKS - 2), False, False)
            weighted_sum(N_BLOCKS - 1, e8s.pop(N_BLOCKS - 1), False, True)

            # S partials ride the idle sync ring; ACT (closest to PSUM,
            # free after the last exp) copies U, sync sends it
            nc.sync.dma_start(OUT_S[:], outs_sb[:])
            nc.scalar.copy(u_sb[:], u_ps[:])
            nc.sync.dma_start(OUT_U[:], u_sb[:])

    nc.compile()
    return nc


def _get_program(full_fac: bool):
    if full_fac not in _cached:
        _cached[full_fac] = _build_program(full_fac)
    return _cached[full_fac]


def kernel(curr_pos, z_curr, z_past, Wq, bq, Wk, bk, Wv, bv, Wo, bo, rel_bias):
    curr_pos = int(np.asarray(curr_pos))
    z_curr = np.asarray(z_curr, dtype=np.float32)
    z_past = np.asarray(z_past, dtype=np.float32)
    Wq = np.asarray(Wq, dtype=np.float32)
    bq = np.asarray(bq, dtype=np.float32)
    Wk = np.asarray(Wk, dtype=np.float32)
    bk = np.asarray(bk, dtype=np.float32)
    Wv = np.asarray(Wv, dtype=np.float32)
    bv = np.asarray(bv, dtype=np.float32)
    Wo = np.asarray(Wo, dtype=np.float32)
    bo = np.asarray(bo, dtype=np.float32)
    rel_bias = np.asarray(rel_bias, dtype=np.float32)

    # ---- host-side O(D^2) prep (f64) ----
    q = z_curr.reshape(-1).astype(np.float64) @ Wq.T.astype(np.float64) + bq
    A = np.zeros((D2, HEADS), np.float64)
    c = np.zeros(HEADS, np.float64)
    for h in range(HEADS):
        sl = slice(h * 2 * HD, (h + 1) * 2 * HD)
        A[:, h] = Wk[sl, :].T.astype(np.float64) @ q[sl]
        c[h] = bk[sl].astype(np.float64) @ q[sl]
    relflat = rel_bias.reshape(2 * REL_MAX + 1, D2).astype(np.float64)
    rb = np.stack(
        [
            relflat[:, h * 2 * HD : (h + 1) * 2 * HD] @ q[h * 2 * HD : (h + 1) * 2 * HD]
            for h in range(HEADS)
        ],
        axis=1,
    )  # [129, 8]
    idx = np.clip(
        curr_pos - L_TOTAL + np.arange(L_TOTAL) + REL_MAX, 0, 2 * REL_MAX
    ).astype(np.int64)

    z8 = np.clip(z_past.reshape(L_TOTAL, D2), -240.0, 240.0).astype(FP8)
    A8 = np.clip(A, -240.0, 240.0).astype(np.float32).astype(FP8)
    a_dr = np.zeros((128, 2, 2, 16), FP8)
    a_dr[:, :, :, 0:8] = A8.reshape(2, 2, 128, HEADS).transpose(2, 0, 1, 3)

    in_maps = []
    facs = []
    for core in range(N_CORES):
        zc = z8[core * L_SHARD : (core + 1) * L_SHARD]
        # zt_b[p, cpair, d, l] = zc[512*b + l, 256*cpair + 128*d + p]
        zt_all = np.ascontiguousarray(
            zc.reshape(N_BLOCKS, 512, 2, 2, 128).transpose(0, 4, 2, 3, 1)
        )
        # zn_b[p, s, d, f] = zc[512*b + 256*s + 128*d + p, f]
        zn_all = np.ascontiguousarray(
            zc.reshape(N_BLOCKS, 2, 2, 128, 512).transpose(0, 3, 1, 2, 4)
        )

        def pk(blob, lo, hi):  # [nblk,128,2,2,512] -> [128,nblk,2,2,512]
            return np.ascontiguousarray(blob[lo:hi].transpose(1, 0, 2, 3, 4))

        def pks(blob, lo, hi):  # -> [nsup,128,4,2,2,512]
            n = (hi - lo) // 4
            return np.ascontiguousarray(
                blob[lo:hi].reshape(n, 4, 128, 2, 2, 512).transpose(
                    0, 2, 1, 3, 4, 5
                )
            )

        idx_c = idx[core * L_SHARD : (core + 1) * L_SHARD]
        base = int(np.bincount(idx_c, minlength=2 * REL_MAX + 1).argmax())
        cb = ((c + rb[base]) * SCALE).astype(np.float32).reshape(HEADS, 1)
        fac = np.ascontiguousarray(
            np.exp((rb[idx_c] - rb[base]) * SCALE).T.astype(BF16)
        )
        facs.append(fac)
        in_maps.append(
            {
                "zt0": np.concatenate(
                    [zt_all[0], a_dr, zt_all[1]], axis=3
                ),
                "zt3": np.ascontiguousarray(
                    zt_all[2:14].reshape(4, 3, 128, 2, 2, 512).transpose(
                        0, 2, 1, 3, 4, 5
                    )
                ),
                "zt1415": np.ascontiguousarray(
                    zt_all[14:16].transpose(1, 0, 2, 3, 4)
                ),
                "zn3": np.ascontiguousarray(
                    zn_all[0:12].reshape(4, 3, 128, 2, 2, 512).transpose(
                        0, 2, 1, 3, 4, 5
                    )
                ),
                "zn1213": np.ascontiguousarray(
                    zn_all[12:14].transpose(1, 0, 2, 3, 4)
                ),
                "zn1415": np.ascontiguousarray(
                    zn_all[14:16].transpose(1, 0, 2, 3, 4)
                ),
                "cb": cb,
            }
        )

    # fast path: correction factors are 1.0 outside block 0 on every core
    full_fac = any(
        not np.all(f[:, 512:] == np.asarray(1.0, BF16)) for f in facs
    )
    facw = L_SHARD if full_fac else 512
    for core, m in enumerate(in_maps):
        cst = np.zeros((8, 9 + facw), BF16)
        cst[:, 0:1] = m.pop("cb").astype(BF16)
        cst[:, 1:9] = np.eye(8, dtype=BF16)
        cst[:, 9:] = facs[core][:, 0:facw]
        m["cst"] = cst
    nc = _get_program(full_fac)
    res = run_bass_kernel_spmd(
        nc, in_maps, list(range(N_CORES)), trace=TRACE, **TRACE_KW
    )
    if TRACE:
        kernel.last_result = res

    U = np.zeros((HEADS, D2), np.float64)
    S = np.zeros(HEADS, np.float64)
    for r in res.results:
        U += np.asarray(r["out_u"], dtype=np.float64)
        S += np.asarray(r["out_s"], dtype=np.float64).sum(axis=1)

    hvec = np.zeros(D2, np.float64)
    for h in range(HEADS):
        sl = slice(h * 2 * HD, (h + 1) * 2 * HD)
        hvec[sl] = Wv[sl, :].astype(np.float64) @ (U[h] / S[h]) + bv[sl]
    out = hvec @ Wo.T.astype(np.float64) + bo
    return out.reshape(DIM, 2).astype(np.float32)
